# revision 1
# baseline (speedup 1.0000x reference)
"""Trainium2 Bass kernel for nn_GaussianLayer: ReflectionPad2d(10) +
depthwise 21x21 Gaussian conv on x:(16,3,512,512) f32.

Strategy
--------
The 21x21 Gaussian kernel is separable (rank-1): W[i,j] = wr[i]*wc[j].
Each (batch, channel) image is blurred with two 1D passes. Reflection
padding is folded into two precomputed 512x512 banded matrices Bv, Bh
(band width 21, edge taps folded by the reflection), so that per image

    y = Bv.T @ x @ Bh       (x, y: 512x512)

On the PE (tensor engine, out = lhsT.T @ rhs, contraction over the
partition dim) both passes use the *image* as the stationary operand,
which absorbs the transposes:

    pass 1: t1 = x.T @ Bv   (lhsT = x chunk  [rows, cols],  rhs = Bv)
    pass 2: y  = t1.T @ Bh  (lhsT = t1 chunk [cols, rows],  rhs = Bh)

Each pass is 4 K-chunks x 4 M-chunks of 128 with banded PSUM
accumulation (per-element has_written semantics). Sharding: pure data
parallel, 2 batches (6 images) per core across 8 cores.
"""

import numpy as np

import concourse.bass as bass
import concourse.mybir as mybir
import concourse.tile as tile
from concourse.bass_utils import run_bass_kernel_spmd

KSIZE = 21
PAD = 10
H = 512
NBATCH = 16
NCH = 3
NCORES = 8
BATCH_PER_CORE = NBATCH // NCORES
IMGS = BATCH_PER_CORE * NCH  # 6 images per core
NCHUNK = H // 128  # 4
XBUFS = 2  # SBUF pool depth for x / t1 / y staging

F32 = mybir.dt.float32
F32R = mybir.dt.float32r

# float32r streams at 1 cycle/row on the PE when the moving dim is >=256
# (fp32 pays 4): widen each banded region to 256 and run the matmuls on
# bitcast-to-f32r operands. Region 0 is widened to overlap every other
# region so its bank-clearing start=True matmul is WAW-ordered first.
USE_F32R = False


def _expand_ranges(ranges):
    out = []
    for j, (n0, n1) in enumerate(ranges):
        if j == 0:
            out.append((0, max(264, n1)))
        else:
            start = max(0, min(n0, H - 256))
            out.append((start, max(start + 256, n1)))
    return out


MAX_WAITS_PER_INST = 1


def _split_multi_waits(nc):
    """Rewrite instructions with >1 sem waits for this toolchain's walrus.

    The walrus codegen here rejects any instruction with more than one
    sync wait ("Too many sync wait commands", CoreV3GenImpl
    setupSyncWait). Surplus waits are moved onto freshly created nop
    instructions on the same engine, inserted immediately before the
    overloaded instruction — engine streams execute in order, so the
    guard is equivalent.
    """
    cur_bb = nc.cur_bb.bb
    for bb in nc.m.functions[0].blocks:
        out = []
        for inst in list(bb.instructions):
            si = inst.sync_info
            waits = list(si.on_wait) if si is not None and si.on_wait else []
            if len(waits) > MAX_WAITS_PER_INST:
                surplus = waits[:-MAX_WAITS_PER_INST]
                keep = waits[-MAX_WAITS_PER_INST:]
                upd = list(si.on_update) if si.on_update else []
                inst.sync_info = mybir.SyncInfo(on_wait=keep, on_update=upd)
                for w in surplus:
                    ni = nc.engines[inst.engine].nop().ins
                    assert cur_bb.instructions[-1] is ni
                    cur_bb.instructions.pop()
                    ni.sync_info = mybir.SyncInfo(on_wait=[w], on_update=[])
                    out.append(ni)
            out.append(inst)
        bb.instructions[:] = out


def _factor_kernel(w2d):
    """Rank-1 factor a (21,21) kernel: w2d[i,j] = wr[i]*wc[j]."""
    u, s, vt = np.linalg.svd(w2d.astype(np.float64))
    wr = u[:, 0] * np.sqrt(s[0])
    wc = vt[0] * np.sqrt(s[0])
    if wr.sum() < 0:
        wr, wc = -wr, -wc
    resid = np.abs(np.outer(wr, wc) - w2d).max()
    scale = max(np.abs(w2d).max(), 1e-30)
    assert resid <= 1e-4 * scale, f"kernel not separable: resid={resid}, scale={scale}"
    return wr, wc


def _band(w1d):
    """(21,) taps -> (512,512) f32 band matrix with reflection folded.

    B[r, n] accumulates every tap of output position n whose reflected
    source row is r:  out[n] = sum_r B[r, n] * x[r].
    """
    b = np.zeros((H, H), np.float64)
    for k in range(KSIZE):
        n = np.arange(H)
        r = n + k - PAD
        r = np.where(r < 0, -r, r)
        r = np.where(r >= H, 2 * H - 2 - r, r)
        np.add.at(b, (r, n), w1d[k])
    return np.ascontiguousarray(b.astype(np.float32))


def _chunk_ranges(b):
    """Nonzero output-column range [n0, n1) of each 128-row chunk of b."""
    ranges = []
    for j in range(NCHUNK):
        nz = np.flatnonzero(np.abs(b[128 * j : 128 * (j + 1)]).max(axis=0) > 0)
        ranges.append((int(nz[0]), int(nz[-1]) + 1))
    return ranges


def _build_program(share_band, rv, rh):
    nc = bass.Bass("TRN2", target_bir_lowering=False, debug=False)
    x = nc.dram_tensor("x", [IMGS, H, H], F32, kind="ExternalInput").ap()
    bv = nc.dram_tensor("bv", [H, H], F32, kind="ExternalInput").ap()
    bh = bv if share_band else nc.dram_tensor("bh", [H, H], F32, kind="ExternalInput").ap()
    y = nc.dram_tensor("y", [IMGS, H, H], F32, kind="ExternalOutput").ap()

    with tile.TileContext(nc) as tc:
        with (
            tc.tile_pool(name="band", bufs=1) as band_pool,
            tc.tile_pool(name="xin", bufs=XBUFS) as xpool,
            tc.tile_pool(name="t1", bufs=XBUFS) as t1pool,
            tc.tile_pool(name="yout", bufs=XBUFS) as ypool,
            tc.tile_pool(name="p1", bufs=4, space="PSUM") as p1pool,
            tc.tile_pool(name="p2", bufs=4, space="PSUM") as p2pool,
        ):
            bv_s = band_pool.tile([128, NCHUNK, H], F32, tag="bv")
            bh_s = (
                bv_s
                if share_band
                else band_pool.tile([128, NCHUNK, H], F32, tag="bh")
            )

            for i in range(IMGS):
                xs = xpool.tile([128, NCHUNK, H], F32, tag="xs")
                for j in range(NCHUNK):
                    nc.sync.dma_start(xs[:, j, :], x[i, 128 * j : 128 * (j + 1), :])
                    if i == 0:
                        # Interleave band loads with the first image so the
                        # first matmul group waits on 2 DMAs, not 8.
                        nc.sync.dma_start(
                            bv_s[:, j, :], bv[128 * j : 128 * (j + 1), :]
                        )
                        if not share_band:
                            nc.sync.dma_start(
                                bh_s[:, j, :], bh[128 * j : 128 * (j + 1), :]
                            )

                # pass 1: t1 = x.T @ Bv  -> [cols, out-rows]
                t1 = t1pool.tile([128, NCHUNK, H], F32, tag="t1")
                for m in range(NCHUNK):
                    p1 = p1pool.tile([128, H], F32, tag="p1")
                    for j in range(NCHUNK):
                        # Banded regions: adjacent chunks overlap, so the
                        # WAW chain forces the start=True matmul first. The
                        # bank-uniformity assert exists only in CoreSim; HW
                        # has_written is per-element.
                        n0, n1 = rv[j]
                        lhs1 = xs[:, j, 128 * m : 128 * (m + 1)]
                        rhs1 = bv_s[:, j, n0:n1]
                        if USE_F32R:
                            lhs1, rhs1 = lhs1.bitcast(F32R), rhs1.bitcast(F32R)
                        nc.tensor.matmul(
                            p1[:, n0:n1],
                            lhs1,
                            rhs1,
                            start=(j == 0),
                            stop=(j == NCHUNK - 1),
                        )
                    if m % 2 == 0:
                        nc.vector.tensor_copy(t1[:, m, :], p1[:])
                    else:
                        nc.scalar.copy(t1[:, m, :], p1[:])

                # pass 2: y = t1.T @ Bh -> [out-rows, out-cols]
                ys = ypool.tile([128, NCHUNK, H], F32, tag="ys")
                for r in range(NCHUNK):
                    p2 = p2pool.tile([128, H], F32, tag="p2")
                    for c in range(NCHUNK):
                        n0, n1 = rh[c]
                        lhs2 = t1[:, c, 128 * r : 128 * (r + 1)]
                        rhs2 = bh_s[:, c, n0:n1]
                        if USE_F32R:
                            lhs2, rhs2 = lhs2.bitcast(F32R), rhs2.bitcast(F32R)
                        nc.tensor.matmul(
                            p2[:, n0:n1],
                            lhs2,
                            rhs2,
                            start=(c == 0),
                            stop=(c == NCHUNK - 1),
                        )
                    if r % 2 == 0:
                        nc.scalar.copy(ys[:, r, :], p2[:])
                    else:
                        nc.vector.tensor_copy(ys[:, r, :], p2[:])
                    nc.sync.dma_start(y[i, 128 * r : 128 * (r + 1), :], ys[:, r, :])

    _split_multi_waits(nc)
    return nc


def _prepare(x, W):
    assert x.shape == (NBATCH, NCH, H, H), x.shape
    assert W.shape == (NCH, 1, KSIZE, KSIZE), W.shape
    w0 = np.asarray(W[0, 0], np.float32)
    for c in range(1, NCH):
        assert np.array_equal(np.asarray(W[c, 0], np.float32), w0), (
            "per-channel kernels differ; single-band path only"
        )
    wr, wc = _factor_kernel(w0)
    bv = _band(wr)
    bh = _band(wc)
    share = bool(np.array_equal(bv, bh))
    return bv, bh, share


def _run(x, W, **spmd_kwargs):
    x = np.ascontiguousarray(np.asarray(x, np.float32))
    bv, bh, share = _prepare(x, W)
    rv = _chunk_ranges(bv)
    rh = _chunk_ranges(bh)

    if USE_F32R:
        rv = _expand_ranges(rv)
        rh = _expand_ranges(rh)
    nc = _build_program(share, rv, rh)

    in_maps = []
    for c in range(NCORES):
        shard = np.ascontiguousarray(
            x[c * BATCH_PER_CORE : (c + 1) * BATCH_PER_CORE].reshape(IMGS, H, H)
        )
        m = {"x": shard, "bv": bv}
        if not share:
            m["bh"] = bh
        in_maps.append(m)

    res = run_bass_kernel_spmd(nc, in_maps, list(range(NCORES)), **spmd_kwargs)
    out = np.empty((NBATCH, NCH, H, H), np.float32)
    for c in range(NCORES):
        out[c * BATCH_PER_CORE : (c + 1) * BATCH_PER_CORE] = res.results[c][
            "y"
        ].reshape(BATCH_PER_CORE, NCH, H, H)
    return out, res


def kernel(x, W):
    return _run(x, W)[0]



# revision 2
# speedup vs baseline: 1.9021x; 1.9021x over previous
"""Trainium2 Bass kernel for nn_GaussianLayer: ReflectionPad2d(10) +
depthwise 21x21 Gaussian conv on x:(16,3,512,512) f32.

Strategy (v2: bf16 wire + PE, banded-strip weights, software-pipelined)
----------------------------------------------------------------------
The 21x21 Gaussian kernel is separable (rank-1): W[i,j] = wr[i]*wc[j].
Each (batch, channel) image is blurred with two 1D passes. Reflection
padding is folded into a 512x512 banded matrix B (band width 21, edge
taps folded by the reflection), so that per image

    y = B.T @ x @ B        (x, y: 512x512, B symmetric for Gaussian)

On the PE (out = lhsT.T @ rhs, contraction over the partition dim) both
passes use the *image* chunk as the stationary operand, which absorbs
the transposes and keeps the moving dim equal to the band's nonzero
output range (~148) instead of the full 512:

    pass 1: t1 = x.T @ B    (lhsT = x chunk,  rhs = B row-chunk strip)
    pass 2: y  = t1.T @ B   (lhsT = t1 chunk, rhs = B row-chunk strip)

Cost levers vs the f32 v1:
  * bf16 operands: PE runs 1 cycle/row vs 4 for f32 (PSUM accum stays
    f32; measured end-to-end max rel err ~4.5e-3 vs the 2e-2 gate).
  * x is pre-cast to bf16 and pre-permuted host-side to [i, p, j, c]
    (p = partition, j = 128-row chunk) so each image loads with ONE
    contiguous DMA; y returns bf16 the same way. Wire traffic per core
    drops 14MB -> ~6.1MB, and the DMA count drops ~56 -> 19 (each DMA
    costs ~650ns of serialized HWDGE time).
  * B is Toeplitz away from the reflection edges, so only 3 strips
    [128, 3, 148] are shipped instead of the dense 512x512 matrix:
    chunk j=1 and j=2 share the interior strip.
  * Pass 1 of image i is issued before pass 2 of image i-1: the PE
    never waits on the PSUM->SBUF staging copies, so it stays
    continuously busy and holds its p-state ramp.

PSUM tiles are [128, 2, 512] (2 banks); each 128-row output chunk
accumulates 4 banded matmuls in its own bank (per-element has_written
semantics make the partial-range start=True safe, as in v1), and one
copy per 2-bank tile casts f32 -> bf16 into SBUF, alternating between
the DVE and Activation engines. Sharding: pure data parallel, 2
batches (6 images) per core across 8 cores.
"""

import numpy as np
import ml_dtypes

import concourse.bass as bass
import concourse.mybir as mybir
import concourse.tile as tile
from concourse.bass_utils import run_bass_kernel_spmd

BF16NP = ml_dtypes.bfloat16

KSIZE = 21
PAD = 10
H = 512
NBATCH = 16
NCH = 3
NCORES = 8
BATCH_PER_CORE = NBATCH // NCORES
IMGS = BATCH_PER_CORE * NCH  # 6 images per core
NCHUNK = H // 128  # 4
SW = 148  # strip width: 128 + (KSIZE - 1)

F32 = mybir.dt.float32
BF16 = mybir.dt.bfloat16

# (strip index, nonzero width, output-column start) for each 128-row
# source chunk j of the band matrix. Chunks 1 and 2 share the interior
# Toeplitz strip.
CHUNK_PLAN = [(0, 138, 0), (1, 148, 118), (1, 148, 246), (2, 138, 374)]

MAX_WAITS_PER_INST = 1


def _split_multi_waits(nc):
    """Rewrite instructions with >1 sem waits for this toolchain's walrus.

    The walrus codegen here rejects any instruction with more than one
    sync wait ("Too many sync wait commands", CoreV3GenImpl
    setupSyncWait). Surplus waits are moved onto freshly created nop
    instructions on the same engine, inserted immediately before the
    overloaded instruction — engine streams execute in order, so the
    guard is equivalent.
    """
    cur_bb = nc.cur_bb.bb
    for bb in nc.m.functions[0].blocks:
        out = []
        for inst in list(bb.instructions):
            si = inst.sync_info
            waits = list(si.on_wait) if si is not None and si.on_wait else []
            if len(waits) > MAX_WAITS_PER_INST:
                surplus = waits[:-MAX_WAITS_PER_INST]
                keep = waits[-MAX_WAITS_PER_INST:]
                upd = list(si.on_update) if si.on_update else []
                inst.sync_info = mybir.SyncInfo(on_wait=keep, on_update=upd)
                for w in surplus:
                    ni = nc.engines[inst.engine].nop().ins
                    assert cur_bb.instructions[-1] is ni
                    cur_bb.instructions.pop()
                    ni.sync_info = mybir.SyncInfo(on_wait=[w], on_update=[])
                    out.append(ni)
            out.append(inst)
        bb.instructions[:] = out
    return nc


def _factor_kernel(w2d):
    """Rank-1 factor a (21,21) kernel: w2d[i,j] = wr[i]*wc[j]."""
    u, s, vt = np.linalg.svd(w2d.astype(np.float64))
    wr = u[:, 0] * np.sqrt(s[0])
    wc = vt[0] * np.sqrt(s[0])
    if wr.sum() < 0:
        wr, wc = -wr, -wc
    resid = np.abs(np.outer(wr, wc) - w2d).max()
    scale = max(np.abs(w2d).max(), 1e-30)
    assert resid <= 1e-4 * scale, f"kernel not separable: resid={resid}, scale={scale}"
    return wr, wc


def _band(w1d):
    """(21,) taps -> (512,512) f64 band matrix with reflection folded.

    B[r, n] accumulates every tap of output position n whose reflected
    source row is r:  out[n] = sum_r B[r, n] * x[r].
    """
    b = np.zeros((H, H), np.float64)
    for k in range(KSIZE):
        n = np.arange(H)
        r = n + k - PAD
        r = np.where(r < 0, -r, r)
        r = np.where(r >= H, 2 * H - 2 - r, r)
        np.add.at(b, (r, n), w1d[k])
    return b


def _strips(b):
    """Extract the 3 distinct [128, *] strips of the banded matrix.

    Strip 0: rows 0..127 (top reflection edge), cols [0, 138).
    Strip 1: rows 128..255, cols [118, 266) — pure Toeplitz interior,
             identical (shifted) to rows 256..383 / cols [246, 394).
    Strip 2: rows 384..511 (bottom edge), cols [374, 512).
    """
    assert np.array_equal(b[256:384, 246:394], b[128:256, 118:266]), (
        "interior band chunks are not translation invariant"
    )
    # Each chunk's nonzeros must lie inside its declared column range.
    assert np.abs(b[0:128, 138:]).max() == 0
    assert np.abs(b[128:256, :118]).max() == 0 and np.abs(b[128:256, 266:]).max() == 0
    assert np.abs(b[256:384, :246]).max() == 0 and np.abs(b[256:384, 394:]).max() == 0
    assert np.abs(b[384:512, :374]).max() == 0
    s = np.zeros((128, 3, SW), np.float32)
    s[:, 0, :138] = b[0:128, 0:138]
    s[:, 1, :148] = b[128:256, 118:266]
    s[:, 2, :138] = b[384:512, 374:512]
    return s.astype(BF16NP)


def _build_program(share_band):
    nc = bass.Bass("TRN2", target_bir_lowering=False, debug=False)
    x = nc.dram_tensor("x", [IMGS, 128, NCHUNK, H], BF16, kind="ExternalInput").ap()
    bs = nc.dram_tensor("bs", [128, 3, SW], BF16, kind="ExternalInput").ap()
    bh = bs if share_band else nc.dram_tensor("bh", [128, 3, SW], BF16, kind="ExternalInput").ap()
    y = nc.dram_tensor("y", [IMGS, 128, NCHUNK, H], BF16, kind="ExternalOutput").ap()

    with tile.TileContext(nc) as tc:
        with (
            tc.tile_pool(name="band", bufs=1) as band_pool,
            tc.tile_pool(name="xin", bufs=IMGS) as xpool,
            tc.tile_pool(name="t1", bufs=2) as t1pool,
            tc.tile_pool(name="yout", bufs=2) as ypool,
            tc.tile_pool(name="p1", bufs=2, space="PSUM") as p1pool,
            tc.tile_pool(name="p2", bufs=2, space="PSUM") as p2pool,
        ):
            bs_s = band_pool.tile([128, 3, SW], BF16, tag="bs")
            nc.sync.dma_start(bs_s[:, :, :], bs[:, :, :])
            if share_band:
                bh_s = bs_s
            else:
                bh_s = band_pool.tile([128, 3, SW], BF16, tag="bh")
                nc.sync.dma_start(bh_s[:, :, :], bh[:, :, :])

            # Preload all 6 images: one contiguous DMA each, no waits.
            xs = []
            for i in range(IMGS):
                xt = xpool.tile([128, NCHUNK, H], BF16, tag="xs")
                nc.sync.dma_start(xt[:, :, :], x[i, :, :, :])
                xs.append(xt)

            copy_engines = [nc.vector.tensor_copy, nc.scalar.copy]

            # One image deep software pipeline: pass1(i) before pass2(i-1)
            # keeps the PE from waiting on the t1 staging copies.
            t1s = [None, None]
            for stage in range(IMGS + 1):
                if stage < IMGS:
                    i = stage
                    t1 = t1pool.tile([128, NCHUNK, H], BF16, tag="t1")
                    t1s[i % 2] = t1
                    for h in range(2):
                        p1 = p1pool.tile([128, 2, H], F32, tag="p1")
                        for mm in range(2):
                            m = 2 * h + mm
                            for j in range(NCHUNK):
                                sj, w, n0 = CHUNK_PLAN[j]
                                nc.tensor.matmul(
                                    p1[:, mm, n0 : n0 + w],
                                    xs[i][:, j, 128 * m : 128 * (m + 1)],
                                    bs_s[:, sj, 0:w],
                                    start=(j == 0),
                                    stop=(j == NCHUNK - 1),
                                )
                        copy_engines[h](t1[:, 2 * h : 2 * h + 2, :], p1[:, :, :])

                if stage >= 1:
                    k = stage - 1
                    t1k = t1s[k % 2]
                    ys = ypool.tile([128, NCHUNK, H], BF16, tag="ys")
                    for h in range(2):
                        p2 = p2pool.tile([128, 2, H], F32, tag="p2")
                        for rr in range(2):
                            r = 2 * h + rr
                            for c in range(NCHUNK):
                                sj, w, n0 = CHUNK_PLAN[c]
                                nc.tensor.matmul(
                                    p2[:, rr, n0 : n0 + w],
                                    t1k[:, c, 128 * r : 128 * (r + 1)],
                                    bh_s[:, sj, 0:w],
                                    start=(c == 0),
                                    stop=(c == NCHUNK - 1),
                                )
                        copy_engines[1 - h](ys[:, 2 * h : 2 * h + 2, :], p2[:, :, :])
                        # Store each half as soon as its copy lands; two
                        # DMAs per image smooth the tail.
                        nc.sync.dma_start(
                            y[k, :, 2 * h : 2 * h + 2, :], ys[:, 2 * h : 2 * h + 2, :]
                        )

    return _split_multi_waits(nc)


def _prepare(x, W):
    assert x.shape == (NBATCH, NCH, H, H), x.shape
    assert W.shape == (NCH, 1, KSIZE, KSIZE), W.shape
    w0 = np.asarray(W[0, 0], np.float32)
    for c in range(1, NCH):
        assert np.array_equal(np.asarray(W[c, 0], np.float32), w0), (
            "per-channel kernels differ; single-band path only"
        )
    wr, wc = _factor_kernel(w0)
    sv = _strips(_band(wr))
    sh = _strips(_band(wc))
    share = bool(np.array_equal(sv, sh))
    return sv, sh, share


def _permute_in(imgs):
    """[IMGS, 512, 512] -> [IMGS, 128, 4, 512] (i, p, j, c) layout."""
    return np.ascontiguousarray(
        imgs.reshape(IMGS, NCHUNK, 128, H).transpose(0, 2, 1, 3)
    )


def _permute_out(y_dev):
    """[IMGS, 128, 4, 512] -> [IMGS, 512, 512]."""
    return y_dev.transpose(0, 2, 1, 3).reshape(IMGS, H, H)


def _run(x, W, **spmd_kwargs):
    x = np.asarray(x, np.float32)
    sv, sh, share = _prepare(x, W)
    nc = _build_program(share)

    in_maps = []
    for c in range(NCORES):
        shard = x[c * BATCH_PER_CORE : (c + 1) * BATCH_PER_CORE].reshape(IMGS, H, H)
        m = {"x": _permute_in(shard.astype(BF16NP)), "bs": sv}
        if not share:
            m["bh"] = sh
        in_maps.append(m)

    res = run_bass_kernel_spmd(nc, in_maps, list(range(NCORES)), **spmd_kwargs)
    out = np.empty((NBATCH, NCH, H, H), np.float32)
    for c in range(NCORES):
        yc = _permute_out(np.asarray(res.results[c]["y"])).astype(np.float32)
        out[c * BATCH_PER_CORE : (c + 1) * BATCH_PER_CORE] = yc.reshape(
            BATCH_PER_CORE, NCH, H, H
        )
    return out, res


def build_for_timing(x, W):
    """Program as run on each core, for the cost-model timeline."""
    _, _, share = _prepare(np.asarray(x, np.float32), W)
    return _build_program(share)


def kernel(x, W):
    return _run(x, W)[0]
